# revision 1
# baseline (speedup 1.0000x reference)
"""GroupedVectorAttention Trainium2 kernel v2 (8-core SPMD, data-parallel).

Reference (N=40000, S=16 neighbors, C=96 channels, G=12 groups):
  query = relu(LN(q @ Wq))   keyf = relu(LN(k @ Wk))   val = v @ Wv
  pos = xyz[idx] - xyz[center];  peb = relu(LN(pos @ Wp1)) @ Wp2
  rel = keyf[idx] - query + peb; valg = val[idx] + peb
  w = softmax_s(relu(LN_G(rel @ Ww1)) @ Ww2)
  out[n, c] = sum_s w[n, s, c//8] * valg[n, s, c]

v2 design (everything fp16 on device; fp32 PSUM accumulation):
  Phase A (duplicated on every core): project k/v for all N rows, write a
  packed table of "superrows": each 512B superrow holds TWO point rows
  [val 96 | bkey 12 | xyz 3 | pad] [same for the odd row].  bkey = keyf@Ww1c
  (the rel@Ww1 product is distributed; keyf itself is never gathered).
  Phase A2: per own point, qpack = [aq 12 | xyz 3 | pad] (query@Ww1c).
  Phase B: per 128-point tile, ONE dma_gather (custom InstDMAGatherAnt)
  fetches the 2048 neighbor superrows (int16 half-indices; the right parity
  is selected on-chip for the 15 small fields and folded into the softmax
  weights for the value field).  The peb MLP runs with block-diagonal
  Wp1 stationaries (no per-neighbor transposes) and the LN rstd is applied
  AFTER the Wp2 matmuls (row scaling commutes), computed from a 3x3 Gram
  quadratic form.  LayerNorms use host-centered weights so LN reduces to
  y * rsqrt(mean(y^2)+eps).
"""

import numpy as np
from contextlib import ExitStack

import concourse.bass as bass
import concourse.bacc as bacc
import concourse.tile as tile
from concourse import mybir
from concourse.bass_utils import run_bass_kernel_spmd
from concourse.masks import make_identity

# ---------------------------------------------------------------------------
# Workaround: this walrus build rejects PE Matmult instructions carrying more
# than one semaphore wait ("Too many sync wait commands").  Split extra waits
# onto NoOp carrier instructions on the same engine queue, placed immediately
# before the matmul, right after Tile's wait-assignment pass.
_orig_postorder = tile.postorder_instruction_blocks
_nop_ctr = [0]


def _split_pe_waits(ordered, *args, **kwargs):
    for bb, insts in list(ordered.items()):
        out = []
        for inst in insts:
            si = getattr(inst, "sync_info", None)
            if (si is not None and si.on_wait and len(si.on_wait) > 1
                    and inst.engine != mybir.EngineType.Pool):
                waits = list(si.on_wait)
                for w in waits[:-1]:
                    _nop_ctr[0] += 1
                    nop = mybir.InstNoOp(name=f"nopw-{_nop_ctr[0]}")
                    nop.engine = inst.engine
                    nop.sync_info = mybir.SyncInfo(on_wait=[w], on_update=[])
                    out.append(nop)
                inst.sync_info = mybir.SyncInfo(
                    on_wait=[waits[-1]], on_update=list(si.on_update)
                )
            out.append(inst)
        ordered[bb] = out
    return _orig_postorder(ordered, *args, **kwargs)


tile.postorder_instruction_blocks = _split_pe_waits

from concourse.vector_clock import ScopedClock as _ScopedClock


def _patched_drain_and_barrier(self, tick_clock, wait_clock):
    probe = self.nc.sync.nop(nofuse=True)
    wait_clock.add_sem_waits(
        probe.ins, _ScopedClock({None: tick_clock.global_clock})
    )
    si = probe.ins.sync_info
    if si is not None and si.on_wait and len(si.on_wait) > 1:
        waits = list(si.on_wait)
        probe.ins.sync_info = mybir.SyncInfo(
            on_wait=waits[:1], on_update=list(si.on_update)
        )
        for w in waits[1:]:
            n2 = self.nc.sync.nop(nofuse=True)
            n2.ins.sync_info = mybir.SyncInfo(on_wait=[w], on_update=[])
    self.nc.sync.drain()
    self.nc.all_engine_barrier()
    popped = self.nc._tile_sem_poison_stack.pop()
    assert popped is self._sem_poison
    self.nc.clear_and_free_semaphores(list(self.sems.allocated().values()))
    self.nc.all_engine_barrier()


tile.TileContext._drain_and_barrier = _patched_drain_and_barrier

P = 128
C = 96
G = 12
S = 16
CG = C // G  # 8
EPS = 1e-5
SRE = 256        # fp16 elems per 512B superrow: [row0 112|pad16|row1 112|pad16]
F16 = mybir.dt.float16
F32 = mybir.dt.float32
I16 = mybir.dt.int16
AX = mybir.AxisListType.X
ALU = mybir.AluOpType
ACTF = mybir.ActivationFunctionType


def _build(NR, NT, debug=False):
    """Per-core Bacc kernel. NR = padded rows per core (mult of 512),
    NT = padded table rows (mult of 512)."""
    assert NR % 512 == 0 and NT % 1024 == 0
    NT2 = NT // 2
    nc = bacc.Bacc(dynamic_dma_scratch_size=65536, num_swdge_queues=2)

    k_full = nc.declare_dram_parameter("k", [NT, C], F16, isOutput=False)
    v_full = nc.declare_dram_parameter("v", [NT, C], F16, isOutput=False)
    xyz_full = nc.declare_dram_parameter("xyz", [NT, 3], F16, isOutput=False)
    q_s = nc.declare_dram_parameter("q", [NR, C], F16, isOutput=False)
    xyzs = nc.declare_dram_parameter("xyzs", [NR, 3], F16, isOutput=False)
    idx16_d = nc.declare_dram_parameter("idx16", [NR, P], I16, isOutput=False)
    par_d = nc.declare_dram_parameter("par", [NR, S], F16, isOutput=False)
    Wq_d = nc.declare_dram_parameter("Wqc", [C, C], F16, isOutput=False)
    Wk_d = nc.declare_dram_parameter("Wkc", [C, C], F16, isOutput=False)
    Wv_d = nc.declare_dram_parameter("Wv", [C, C], F16, isOutput=False)
    Ww1_d = nc.declare_dram_parameter("Ww1c", [C, G], F16, isOutput=False)
    Wp1blk_d = nc.declare_dram_parameter("Wp1blk", [3 * S, S * C], F16,
                                         isOutput=False)
    MqB_d = nc.declare_dram_parameter("MqB", [3 * S, 3 * S], F16, isOutput=False)
    Wp2_d = nc.declare_dram_parameter("Wp2", [C, C], F16, isOutput=False)
    Wp2w1_d = nc.declare_dram_parameter("Wp2w1", [C, G], F16, isOutput=False)
    Ww2B_d = nc.declare_dram_parameter("Ww2B", [8 * G, 8 * G], F16, isOutput=False)
    out = nc.declare_dram_parameter("out", [NR, C], F32, isOutput=True)

    packed = nc.dram_tensor("packed", [NT2, SRE], F16)
    qpack = nc.dram_tensor("qpack", [NR, 16], F16)

    with ExitStack() as ctx:
        tc = ctx.enter_context(tile.TileContext(nc))
        consts = ctx.enter_context(tc.tile_pool(name="consts", bufs=1))
        # PSUM pools (8 banks: 2+2+2+2)
        pp_y = ctx.enter_context(tc.tile_pool(name="pp_y", bufs=2, space="PSUM"))
        pp_v = ctx.enter_context(tc.tile_pool(name="pp_v", bufs=2, space="PSUM"))
        pp_tp = ctx.enter_context(tc.tile_pool(name="pp_tp", bufs=2, space="PSUM"))
        pp_w = ctx.enter_context(tc.tile_pool(name="pp_w", bufs=2, space="PSUM"))
        # SBUF pools
        sb_in = ctx.enter_context(tc.tile_pool(name="sb_in", bufs=3))
        sb_t = ctx.enter_context(tc.tile_pool(name="sb_t", bufs=3))
        sb_st = ctx.enter_context(tc.tile_pool(name="sb_st", bufs=3))
        sb_sm = ctx.enter_context(tc.tile_pool(name="sb_sm", bufs=4))
        sb_g = ctx.enter_context(tc.tile_pool(name="sb_g", bufs=5))
        sb_b = ctx.enter_context(tc.tile_pool(name="sb_b", bufs=3))

        ident = consts.tile([P, P], F16)
        make_identity(nc, ident[:])
        epst = consts.tile([P, 1], F32)
        nc.vector.memset(epst[:], EPS)

        def load_const(name, dram, shape):
            t = consts.tile(shape, F16, tag=name)
            nc.sync.dma_start(out=t[:], in_=dram[:])
            return t

        wq_sb = load_const("wq", Wq_d, [C, C])
        wk_sb = load_const("wk", Wk_d, [C, C])
        wv_sb = load_const("wv", Wv_d, [C, C])
        ww1_sb = load_const("ww1", Ww1_d, [C, G])
        wp1b_sb = load_const("wp1b", Wp1blk_d, [3 * S, S * C])
        mqb_sb = load_const("mqb", MqB_d, [3 * S, 3 * S])
        wp2_sb = load_const("wp2", Wp2_d, [C, C])
        wp2w1_sb = load_const("wp2w1", Wp2w1_d, [C, G])
        ww2b_sb = load_const("ww2b", Ww2B_d, [8 * G, 8 * G])

        def proj_ln_relu(src_dram, r0, w_sb, tagpfx):
            """Load 512 rows of src (row = r0+p*4+a), project through w_sb,
            LN+relu -> returns fp16 [P, 4, C] tile (keyf/query rows)."""
            xt = sb_in.tile([P, 4, C], F16, tag="xt")
            nc.sync.dma_start(
                out=xt[:],
                in_=src_dram[r0:r0 + 512, :].rearrange("(p a) c -> p a c", a=4))
            xtp = pp_tp.tile([C, 512], F16, tag="tp")
            for j in range(4):
                nc.tensor.transpose(out=xtp[:, j * P:(j + 1) * P], in_=xt[:, j, :],
                                    identity=ident[:])
            xT = sb_t.tile([C, 512], F16, tag=tagpfx + "xT")
            nc.vector.tensor_copy(out=xT[:], in_=xtp[:])
            y = pp_y.tile([P, 4, C], F32, tag="y")
            for j in range(4):
                nc.tensor.matmul(out=y[:, j, :], lhsT=xT[:, j * P:(j + 1) * P],
                                 rhs=w_sb[:], start=True, stop=True)
            sq = sb_sm.tile([P, 4, C], F16, tag="sqA")
            nc.scalar.activation(out=sq[:], in_=y[:], func=ACTF.Square)
            ssq = sb_sm.tile([P, 4], F32, tag="ssqA")
            nc.vector.tensor_reduce(out=ssq[:], in_=sq[:], axis=AX, op=ALU.add)
            sd = sb_sm.tile([P, 4], F32, tag="sdA")
            nc.scalar.activation(out=sd[:], in_=ssq[:], func=ACTF.Sqrt,
                                 scale=1.0 / C, bias=epst[:])
            rstd = sb_sm.tile([P, 4], F32, tag="rstdA")
            nc.vector.reciprocal(out=rstd[:], in_=sd[:])
            fk = sb_sm.tile([P, 4, C], F16, tag="fkA")
            for j in range(4):
                nc.scalar.activation(out=fk[:, j, :], in_=y[:, j, :],
                                     func=ACTF.Relu, scale=rstd[:, j:j + 1])
            return fk

        def bkey_of(fk, tagpfx):
            """fk [P, 4, C] fp16 -> bkey psum [P, 4, G] f32 (fk @ Ww1c)."""
            ftp = pp_tp.tile([C, 512], F16, tag="tp")
            for j in range(4):
                nc.tensor.transpose(out=ftp[:, j * P:(j + 1) * P], in_=fk[:, j, :],
                                    identity=ident[:])
            fT = sb_t.tile([C, 512], F16, tag=tagpfx + "fT")
            nc.vector.tensor_copy(out=fT[:], in_=ftp[:])
            bk = pp_w.tile([P, 4, G], F32, tag="w")
            for j in range(4):
                nc.tensor.matmul(out=bk[:, j, :], lhsT=fT[:, j * P:(j + 1) * P],
                                 rhs=ww1_sb[:], start=True, stop=True)
            return bk

        # ---------------- Phase A: packed superrow table -----------------------
        for b in range(NT // 512):
            r0 = b * 512
            sr0 = r0 // 2
            stg = sb_st.tile([P, 2, SRE], F16, tag="stg")
            stgv = stg[:].rearrange("p a (o x) -> p a o x", o=2)  # [P,2,2,128]
            nc.vector.memset(stgv[:, :, :, 111:128], 0.0)
            nc.sync.dma_start(
                out=stgv[:, :, :, 108:111],
                in_=xyz_full[r0:r0 + 512, :].rearrange("(p a o) c -> p a o c",
                                                       a=2, o=2))

            fk = proj_ln_relu(k_full, r0, wk_sb, "k")
            bk = bkey_of(fk, "k")
            nc.vector.tensor_copy(
                out=stgv[:, :, :, 96:108],
                in_=bk[:].rearrange("p (a o) g -> p a o g", o=2))

            vt = sb_in.tile([P, 4, C], F16, tag="xt")
            nc.sync.dma_start(
                out=vt[:],
                in_=v_full[r0:r0 + 512, :].rearrange("(p a) c -> p a c", a=4))
            vtp = pp_tp.tile([C, 512], F16, tag="tp")
            for j in range(4):
                nc.tensor.transpose(out=vtp[:, j * P:(j + 1) * P], in_=vt[:, j, :],
                                    identity=ident[:])
            vT = sb_t.tile([C, 512], F16, tag="vT")
            nc.vector.tensor_copy(out=vT[:], in_=vtp[:])
            yv = pp_v.tile([P, 4, C], F32, tag="v")
            for j in range(4):
                nc.tensor.matmul(out=yv[:, j, :], lhsT=vT[:, j * P:(j + 1) * P],
                                 rhs=wv_sb[:], start=True, stop=True)
            nc.vector.tensor_copy(
                out=stgv[:, :, :, 0:96],
                in_=yv[:].rearrange("p (a o) c -> p a o c", o=2))

            nc.sync.dma_start(
                out=packed[sr0:sr0 + 256, :].rearrange("(p a) e -> p a e", a=2),
                in_=stg[:])

        # ---------------- Phase A2: qpack [aq 12 | xyz 3 | pad] ----------------
        for b in range(NR // 512):
            r0 = b * 512
            qstg = sb_st.tile([P, 4, 16], F16, tag="qstg")
            nc.vector.memset(qstg[:, :, 15:16], 0.0)
            nc.sync.dma_start(
                out=qstg[:, :, 12:15],
                in_=xyzs[r0:r0 + 512, :].rearrange("(p a) c -> p a c", a=4))
            fq = proj_ln_relu(q_s, r0, wq_sb, "q")
            aq = bkey_of(fq, "q")
            nc.vector.tensor_copy(out=qstg[:, :, 0:12], in_=aq[:])
            nc.sync.dma_start(
                out=qpack[r0:r0 + 512, :].rearrange("(p a) e -> p a e", a=4),
                in_=qstg[:])

        # ---------------- Phase B: per 128-point tile --------------------------
        for t in range(NR // P):
            r0 = t * P
            qp = sb_sm.tile([P, 16], F16, tag="qp")
            nc.sync.dma_start(out=qp[:], in_=qpack[r0:r0 + P, :])
            pr = sb_sm.tile([P, S], F16, tag="pr")
            nc.sync.dma_start(out=pr[:], in_=par_d[r0:r0 + P, :])
            ixt = sb_sm.tile([P, P], I16, tag="ixt")
            nc.sync.dma_start(out=ixt[:], in_=idx16_d[r0:r0 + P, :])
            Gt = sb_g.tile([P, S, SRE], F16, tag="G")
            nc.gpsimd.dma_gather(
                out_ap=Gt[:], in_ap=packed[:, :], idxs_ap=ixt[:],
                num_idxs=P * S, num_idxs_reg=P * S, elem_size=SRE,
                single_packet=False, queue_num=t % 2)
            Gpair = Gt[:].rearrange("p s (o x) -> p s o x", o=2)

            # parity-select the 15 small fields: [bkey 12 | xyz 3]
            prb15 = pr[:].rearrange("p (s o) -> p s o", o=1).broadcast_to([P, S, 15])
            d15 = sb_sm.tile([P, S, 15], F16, tag="d15")
            nc.vector.tensor_tensor(out=d15[:], in0=Gpair[:, :, 1, 96:111],
                                    in1=Gpair[:, :, 0, 96:111], op=ALU.subtract)
            nc.vector.tensor_tensor(out=d15[:], in0=d15[:], in1=prb15,
                                    op=ALU.mult)
            sel = sb_sm.tile([P, S, 15], F16, tag="sel")
            nc.vector.tensor_tensor(out=sel[:], in0=Gpair[:, :, 0, 96:111],
                                    in1=d15[:], op=ALU.add)

            # pos and its transpose
            ps = sb_sm.tile([P, S, 3], F16, tag="ps")
            nc.vector.tensor_tensor(
                out=ps[:], in0=sel[:, :, 12:15],
                in1=qp[:, 12:15].rearrange("p (o c) -> p o c", o=1)
                    .broadcast_to([P, S, 3]),
                op=ALU.subtract)
            posTp = pp_tp.tile([3 * S, P], F16, tag="tp")
            nc.tensor.transpose(out=posTp[:], in_=ps[:].rearrange("p s c -> p (s c)"),
                                identity=ident[:])
            posT = sb_t.tile([3 * S, P], F16, tag="posT")
            nc.vector.tensor_copy(out=posT[:], in_=posTp[:])

            # rstd via 3x3 Gram quadratic form: ssq = sum_c (pos@Wp1c)^2
            qf = pp_w.tile([P, 3 * S], F32, tag="w")
            nc.tensor.matmul(out=qf[:], lhsT=posT[:], rhs=mqb_sb[:],
                             start=True, stop=True)
            s2 = sb_sm.tile([P, S, 3], F16, tag="s2")
            nc.vector.tensor_tensor(
                out=s2[:], in0=ps[:],
                in1=qf[:].rearrange("p (s c) -> p s c", c=3), op=ALU.mult)
            ssqp = sb_sm.tile([P, S], F32, tag="ssqp")
            nc.vector.tensor_reduce(out=ssqp[:], in_=s2[:], axis=AX, op=ALU.add)
            sdp = sb_sm.tile([P, S], F32, tag="sdp")
            nc.scalar.activation(out=sdp[:], in_=ssqp[:], func=ACTF.Sqrt,
                                 scale=1.0 / C, bias=epst[:])
            rstd = sb_sm.tile([P, S], F32, tag="rstd")
            nc.vector.reciprocal(out=rstd[:], in_=sdp[:])
            rstdb = rstd[:].rearrange("p (s o) -> p s o", o=1)

            # pu matmuls (block-diag Wp1 stationaries) + relu -> pLT
            pLT = sb_b.tile([C, S, P], F16, tag="pLT")
            for g4 in range(4):
                puP = pp_y.tile([C, 4, P], F32, tag="y")
                for j in range(4):
                    s = g4 * 4 + j
                    nc.tensor.matmul(out=puP[:, j, :],
                                     lhsT=wp1b_sb[:, s * C:(s + 1) * C],
                                     rhs=posT[:], start=True, stop=True)
                nc.scalar.activation(out=pLT[:, g4 * 4:(g4 + 1) * 4, :],
                                     in_=puP[:], func=ACTF.Relu)

            # pwa = relu(pu) @ (Wp2@Ww1c)  [the 12-dim weight-branch pos term]
            pwaP = pp_w.tile([P, S, G], F32, tag="w")
            for s in range(S):
                nc.tensor.matmul(out=pwaP[:, s, :], lhsT=pLT[:, s, :],
                                 rhs=wp2w1_sb[:], start=True, stop=True)

            # weight branch: yt = (bkey_g - aq) + rstd*pwa ; LN_G ; relu
            yt = sb_sm.tile([P, S, G], F16, tag="yt")
            nc.vector.tensor_tensor(
                out=yt[:], in0=sel[:, :, 0:12],
                in1=qp[:, 0:12].rearrange("p (o c) -> p o c", o=1)
                    .broadcast_to([P, S, G]),
                op=ALU.subtract)
            tyr = sb_sm.tile([P, S, G], F16, tag="tyr")
            nc.vector.tensor_tensor(out=tyr[:], in0=pwaP[:],
                                    in1=rstdb.broadcast_to([P, S, G]), op=ALU.mult)
            nc.vector.tensor_tensor(out=yt[:], in0=yt[:], in1=tyr[:], op=ALU.add)
            sqg = sb_sm.tile([P, S, G], F16, tag="sqg")
            nc.scalar.activation(out=sqg[:], in_=yt[:], func=ACTF.Square)
            ssqg = sb_sm.tile([P, S], F32, tag="ssqg")
            nc.vector.tensor_reduce(out=ssqg[:], in_=sqg[:], axis=AX, op=ALU.add)
            sdg = sb_sm.tile([P, S], F32, tag="sdg")
            nc.scalar.activation(out=sdg[:], in_=ssqg[:], func=ACTF.Sqrt,
                                 scale=1.0 / G, bias=epst[:])
            rsg = sb_sm.tile([P, S], F32, tag="rsg")
            nc.vector.reciprocal(out=rsg[:], in_=sdg[:])
            yh = sb_sm.tile([P, S, G], F16, tag="yh")
            nc.vector.tensor_tensor(
                out=yh[:], in0=yt[:],
                in1=rsg[:].rearrange("p (s o) -> p s o", o=1)
                    .broadcast_to([P, S, G]),
                op=ALU.mult)
            nc.vector.tensor_scalar_max(out=yh[:], in0=yh[:], scalar1=0.0)

            # z = yh @ kron(I8, Ww2) -> e = exp(z)
            yflat = yh[:].rearrange("p s g -> p (s g)")
            yT = sb_t.tile([C, 2, P], F16, tag="yT")
            for h in range(2):
                yhTp = pp_tp.tile([C, P], F16, tag="tp")
                nc.tensor.transpose(out=yhTp[:], in_=yflat[:, h * C:(h + 1) * C],
                                    identity=ident[:])
                nc.scalar.copy(out=yT[:, h, :], in_=yhTp[:])
            zP = pp_w.tile([P, 2, C], F32, tag="w")
            for h in range(2):
                nc.tensor.matmul(out=zP[:, h, :], lhsT=yT[:, h, :],
                                 rhs=ww2b_sb[:], start=True, stop=True)
            e = sb_sm.tile([P, S, G], F16, tag="e")
            nc.scalar.activation(out=e[:].rearrange("p s g -> p (s g)"),
                                 in_=zP[:].rearrange("p a c -> p (a c)"),
                                 func=ACTF.Exp)
            es = sb_sm.tile([P, G], F32, tag="es")
            nc.vector.tensor_reduce(out=es[:], in_=e[:].rearrange("p s g -> p g s"),
                                    axis=AX, op=ALU.add)
            rq = sb_sm.tile([P, G], F32, tag="rq")
            nc.vector.reciprocal(out=rq[:], in_=es[:])

            # parity-masked weights for the value field + u for the peb field
            ep = sb_sm.tile([P, S, 2, G], F16, tag="ep")
            nc.vector.tensor_tensor(
                out=ep[:, :, 1, :], in0=e[:],
                in1=pr[:].rearrange("p (s o) -> p s o", o=1).broadcast_to([P, S, G]),
                op=ALU.mult)
            nc.vector.tensor_tensor(out=ep[:, :, 0, :], in0=e[:],
                                    in1=ep[:, :, 1, :], op=ALU.subtract)
            u = sb_sm.tile([P, S, G], F16, tag="u")
            nc.vector.tensor_tensor(out=u[:], in0=e[:],
                                    in1=rstdb.broadcast_to([P, S, G]), op=ALU.mult)

            # weighted sums: macc = sum of e'*val(parity) and u*pebraw
            macc = sb_b.tile([P, S, C], F16, tag="macc")
            nc.vector.tensor_tensor(
                out=macc[:].rearrange("p s (g o) -> p s g o", o=CG),
                in0=Gpair[:, :, 0, 0:96].rearrange("p s (g o) -> p s g o", o=CG),
                in1=ep[:, :, 0, :].rearrange("p s (g o) -> p s g o", o=1)
                    .broadcast_to([P, S, G, CG]),
                op=ALU.mult)
            m1b = sb_b.tile([P, S, C], F16, tag="m1b")
            nc.vector.tensor_tensor(
                out=m1b[:].rearrange("p s (g o) -> p s g o", o=CG),
                in0=Gpair[:, :, 1, 0:96].rearrange("p s (g o) -> p s g o", o=CG),
                in1=ep[:, :, 1, :].rearrange("p s (g o) -> p s g o", o=1)
                    .broadcast_to([P, S, G, CG]),
                op=ALU.mult)
            nc.vector.tensor_tensor(out=macc[:], in0=macc[:], in1=m1b[:],
                                    op=ALU.add)

            m2 = sb_b.tile([P, S, C], F16, tag="m2")
            for g4 in range(4):
                pebP = pp_v.tile([P, 4, C], F32, tag="v")
                for j in range(4):
                    s = g4 * 4 + j
                    nc.tensor.matmul(out=pebP[:, j, :], lhsT=pLT[:, s, :],
                                     rhs=wp2_sb[:], start=True, stop=True)
                nc.vector.tensor_tensor(
                    out=m2[:, g4 * 4:(g4 + 1) * 4, :]
                        .rearrange("p s (g o) -> p s g o", o=CG),
                    in0=pebP[:].rearrange("p s (g o) -> p s g o", o=CG),
                    in1=u[:, g4 * 4:(g4 + 1) * 4, :]
                        .rearrange("p s (g o) -> p s g o", o=1)
                        .broadcast_to([P, 4, G, CG]),
                    op=ALU.mult)
            nc.vector.tensor_tensor(out=macc[:], in0=macc[:], in1=m2[:],
                                    op=ALU.add)

            for hw_ in (8, 4, 2, 1):
                nc.vector.tensor_tensor(out=macc[:, 0:hw_, :],
                                        in0=macc[:, 0:hw_, :],
                                        in1=macc[:, hw_:2 * hw_, :], op=ALU.add)
            fo = sb_sm.tile([P, C], F32, tag="fo")
            nc.vector.tensor_tensor(
                out=fo[:].rearrange("p (g o) -> p g o", o=CG),
                in0=macc[:, 0, :].rearrange("p (g o) -> p g o", o=CG),
                in1=rq[:].rearrange("p (g o) -> p g o", o=1).broadcast_to([P, G, CG]),
                op=ALU.mult)
            nc.sync.dma_start(out=out[r0:r0 + P, :], in_=fo[:])

    nc.finalize()
    return nc


def _center(W):
    """Remove the mean over the output axis (last)."""
    W = np.asarray(W, np.float64)
    return (W - W.mean(axis=-1, keepdims=True)).astype(np.float32)


def _prep_host(q, k, v, xyz, reference_index,
               Wq, bq, gq, betaq, Wk, bk, gk, betak, Wv, bv,
               Wp1, bp1, gp, betap, Wp2, bp2, Ww1, bw1, gw, betaw, Ww2, bw2,
               n_cores):
    for name, arr, val in [
        ("bq", bq, 0), ("gq", gq, 1), ("betaq", betaq, 0),
        ("bk", bk, 0), ("gk", gk, 1), ("betak", betak, 0),
        ("bv", bv, 0), ("bp1", bp1, 0), ("gp", gp, 1), ("betap", betap, 0),
        ("bp2", bp2, 0), ("bw1", bw1, 0), ("gw", gw, 1), ("betaw", betaw, 0),
        ("bw2", bw2, 0),
    ]:
        if not np.allclose(np.asarray(arr), val, atol=1e-6):
            raise NotImplementedError(f"non-trivial {name} not supported")

    N = q.shape[0]
    NR = ((N // n_cores) + 511) // 512 * 512
    NT = (N + 1023) // 1024 * 1024

    def padT(a, rows, dtype=np.float16):
        out = np.zeros((rows, a.shape[1]), dtype=dtype)
        out[:a.shape[0]] = np.asarray(a)
        return out

    k_pad = padT(k, NT)
    v_pad = padT(v, NT)
    xyz_pad = padT(xyz, NT)

    Wq32 = _center(Wq)
    Wk32 = _center(Wk)
    Ww1c = _center(Ww1)
    Wp1c = _center(Wp1)                       # [3, C]
    Wp1c16 = Wp1c.astype(np.float16)
    M3 = (Wp1c16.astype(np.float32) @ Wp1c16.astype(np.float32).T)  # [3,3]
    MqB = np.kron(np.eye(S, dtype=np.float32), M3)                  # [48,48]
    Wp1blk = np.zeros((3 * S, S * C), np.float32)
    for s in range(S):
        Wp1blk[3 * s:3 * s + 3, s * C:(s + 1) * C] = Wp1c
    weights = {
        "Wqc": Wq32.astype(np.float16),
        "Wkc": Wk32.astype(np.float16),
        "Wv": np.asarray(Wv, np.float32).astype(np.float16),
        "Ww1c": Ww1c.astype(np.float16),
        "Wp1blk": Wp1blk.astype(np.float16),
        "MqB": MqB.astype(np.float16),
        "Wp2": np.asarray(Wp2, np.float32).astype(np.float16),
        "Wp2w1": (np.asarray(Wp2, np.float32) @ Ww1c).astype(np.float16),
        "Ww2B": np.kron(np.eye(8, dtype=np.float32),
                        np.asarray(Ww2, np.float32)).astype(np.float16),
    }

    per_core = N // n_cores
    assert per_core * n_cores == N
    ref = np.asarray(reference_index, np.int64)
    in_maps = []
    for i in range(n_cores):
        lo, hi = i * per_core, (i + 1) * per_core
        rsl = ref[lo:hi]                       # [per_core, S]
        nt_tiles = NR // P
        # idx16[t*128+p-like rows, 128]: per 128-row tile, int16 half-indices
        # ordered so gather pair j = s*128+p -> idxs[(j%16) within 16-part
        # block replicated 8x, j//16].
        idx16 = np.zeros((NR, P), np.int16)
        par = np.zeros((NR, S), np.float16)
        half = np.zeros((NR, S), np.int16)
        half[:per_core] = (rsl >> 1).astype(np.int16)
        par[:per_core] = (rsl & 1).astype(np.float16)
        for t in range(nt_tiles):
            blk = half[t * P:(t + 1) * P]          # [128, S]
            lin = blk.T.reshape(-1)                # j = s*128+p
            i16 = lin.reshape(P, 16).T             # [16, 128]
            idx16[t * P:(t + 1) * P] = np.tile(i16, (8, 1))
        m = {
            "k": k_pad, "v": v_pad, "xyz": xyz_pad,
            "q": padT(q[lo:hi], NR),
            "xyzs": padT(xyz[lo:hi], NR),
            "idx16": idx16,
            "par": par,
        }
        m.update(weights)
        in_maps.append(m)
    return in_maps, NR, NT, per_core


_CACHE = {}


def kernel(**inputs):
    n_cores = 8
    in_maps, NR, NT, per_core = _prep_host(n_cores=n_cores, **inputs)
    key = (NR, NT)
    if key not in _CACHE:
        _CACHE[key] = _build(NR, NT)
    nc = _CACHE[key]
    res = run_bass_kernel_spmd(nc, in_maps, list(range(n_cores)))
    outs = [res.results[i]["out"][:per_core] for i in range(n_cores)]
    return np.ascontiguousarray(np.concatenate(outs, axis=0), dtype=np.float32)

